# revision 1
# baseline (speedup 1.0000x reference)
"""Multi-head attention (dense_transformer) Trainium2 Bass kernel.

Problem: x[8, 512, 32, 32]; per-batch 1x1-conv QKV projections, 8-head
attention over N=H*W=1024 positions (head_dim 64), output projection,
residual. Sharding: data-parallel over batch B=8 across the 8 cores —
one batch element per core, no collectives.

Per-core dataflow (all matmul inputs bf16, accumulation fp32):
  - Host pre-transposes weights to [c, o] layout and pre-casts to bf16.
  - Q, K in [c, n] layout: Q[ot] = WqT[ct].T @ x16[ct] (+bq).
  - V kept transposed: VT[jt][n, o] = x16[:, jt].T @ WvT (+bv), stored
    per-head with a ones column appended: [128, 8 heads, 65].
  - S^T[j, i] = K_h.T Q_h per head: j on partitions -> AV matmul needs
    no transposes anywhere. exp via ScalarE with the 1/sqrt(64) scale
    folded in; softmax denominator comes from the VT ones column during
    the AV matmul (PSUM row 64); normalization = reciprocal + DRAM-
    bounce partition broadcast + VectorE multiply.
  - out = WoT.T @ O + (x32 + bo prefolded), DMA'd out in fp32.

PSUM (8 banks) is phase-scoped: projections use a 4-buf half-bank pool
that closes before the AV-accumulator pool opens in the same banks.
"""

import sys

if "/opt/trn_rl_repo" not in sys.path:
    sys.path.insert(0, "/opt/trn_rl_repo")

import numpy as np
import ml_dtypes

import concourse.bass as bass
import concourse.mybir as mybir
from concourse.tile import TileContext

DIM = 512
NH = 8
HD = 64
N = 1024
P = 128
CT = DIM // P  # 4 c-tiles of 128 channels
JT = N // P    # 8 j-tiles of 128 positions
F32 = mybir.dt.float32
BF16 = mybir.dt.bfloat16
AOP = mybir.AluOpType
EXP = mybir.ActivationFunctionType.Exp


class FixedTileContext(TileContext):
    """Works around a walrus/bass snapshot mismatch: this walrus build
    accepts only one sync-wait command per instruction, but Tile's wait
    assigner happily attaches several. After scheduling, excess waits on
    any instruction are peeled off onto same-engine NOPs inserted right
    before it (same blocking semantics: the engine executes in order)."""

    MAX_WAITS = 1
    MAX_WAITS_DATA = 1
    _wsplit_ctr = 0

    def _split_sync_waits(self):
        seq_only = mybir.SEQUENCER_ONLY_OPCODES
        for fn in self.nc.m.functions:
            for blk in fn.blocks:
                insts = list(blk.instructions)
                out = []
                for inst in insts:
                    si = inst.sync_info
                    limit = (
                        self.MAX_WAITS
                        if inst.opcode in seq_only
                        else self.MAX_WAITS_DATA
                    )
                    if si is not None and len(si.on_wait) > limit:
                        waits = list(si.on_wait)
                        movers = waits[:-limit]
                        keep = waits[-limit:]
                        del si.on_wait[:]
                        for w in keep:
                            si.on_wait.append(w)
                        for w in movers:
                            FixedTileContext._wsplit_ctr += 1
                            nop = mybir.InstNoOp(
                                name=f"wsplit-{FixedTileContext._wsplit_ctr}",
                                ins=[],
                                outs=[],
                            )
                            nop.engine = inst.engine
                            nop.sync_info = mybir.SyncInfo(on_wait=[w], on_update=[])
                            out.append(nop)
                    out.append(inst)
                if len(out) != len(insts):
                    del blk.instructions[:]
                    for i in out:
                        blk.add_instruction(i)

    split_on_exit = True

    def __exit__(self, *exc):
        ret = super().__exit__(*exc)
        if exc[0] is None and self.split_on_exit:
            self._split_sync_waits()
        return ret


def build_nc(split_waits=True):
    nc = bass.Bass()

    x32d = nc.dram_tensor("x32", [DIM, N], F32, kind="ExternalInput")
    x16d = nc.dram_tensor("x16", [DIM, N], BF16, kind="ExternalInput")
    wqd = nc.dram_tensor("wqt", [DIM, DIM], BF16, kind="ExternalInput")
    wkd = nc.dram_tensor("wkt", [DIM, DIM], BF16, kind="ExternalInput")
    wvd = nc.dram_tensor("wvt", [DIM, DIM], BF16, kind="ExternalInput")
    wod = nc.dram_tensor("wot", [DIM, DIM], BF16, kind="ExternalInput")
    bqd = nc.dram_tensor("bq", [DIM], F32, kind="ExternalInput")
    bkd = nc.dram_tensor("bk", [DIM], F32, kind="ExternalInput")
    bvd = nc.dram_tensor("bv", [DIM], F32, kind="ExternalInput")
    bod = nc.dram_tensor("bo", [DIM], F32, kind="ExternalInput")
    outd = nc.dram_tensor("out", [DIM, N], F32, kind="ExternalOutput")

    FixedTileContext.split_on_exit = split_waits
    with FixedTileContext(nc) as tc:
        with (
            tc.tile_pool(name="persist", bufs=1) as persist,
            tc.tile_pool(name="ppool", bufs=32) as ppool,
            tc.tile_pool(name="small", bufs=3) as small,
            tc.tile_pool(name="otile", bufs=8) as otile,
            tc.tile_pool(name="dram", bufs=1, space="DRAM") as dram,
            tc.tile_pool(name="psS", bufs=2, space="PSUM") as psS_pool,
        ):
            # weights/biases ride ScalarE's DMA queues (ScalarE is idle
            # until the first exp) so they don't serialize behind the x
            # loads on SP's queues
            def load_w(wd, name):
                wr = wd.rearrange("(t p) o -> t p o", p=P)
                ws = []
                for t in range(CT):
                    wt = persist.tile(
                        [P, DIM], BF16, tag=f"{name}_{t}", name=f"{name}_{t}"
                    )
                    nc.scalar.dma_start(out=wt, in_=wr[t])
                    ws.append(wt)
                return ws

            def load_b(bd, name):
                bt = persist.tile([P, CT], F32, tag=name, name=name)
                nc.scalar.dma_start(out=bt, in_=bd.rearrange("(t p) -> p t", p=P))
                return bt

            # S^T + exp for one head pair. Emission alternates PE row
            # groups 0-63 / 64-127 between consecutive matmuls so the
            # hardware overlaps them (per-subarray concurrency) even
            # though K=64 only half-fills the array.
            def s_phase(pair):
                P16 = {}
                for jt in range(JT):
                    tiles = {}

                    def smm(h2, ih):
                        base = 64 * h2
                        nc.tensor.matmul(
                            tiles[h2][:, ih * 512 : (ih + 1) * 512],
                            lhsT=K[pair][base : base + 64, jt * P : (jt + 1) * P],
                            rhs=Q[pair][base : base + 64, ih * 512 : (ih + 1) * 512],
                            start=True,
                            stop=True,
                        )

                    tiles[0] = psS_pool.tile([P, N], F32, tag="psS", name="psS")
                    smm(0, 0)
                    tiles[1] = psS_pool.tile([P, N], F32, tag="psS", name="psS")
                    smm(1, 0)
                    smm(0, 1)
                    smm(1, 1)
                    for h2 in range(2):
                        pt = ppool.tile([P, N], BF16, tag="p16", name="p16")
                        nc.scalar.activation(pt, tiles[h2], EXP, scale=0.125)
                        P16[(jt, h2)] = pt
                return P16

            def p16_slice(P16, jt, h2, ih):
                return P16[(jt, h2)][:, ih * 512 : (ih + 1) * 512]

            # AV matmul + softmax normalization for one head pair. The raw
            # head output is copied out of PSUM right away (frees the psO
            # slot for the next head's AV); the DRAM-bounce broadcast and
            # the normalize multiply then run off the critical PSUM path.
            def av_phase(pair, P16, psO_pool, O16, rdram):
                last_pair = pair == NH // 2 - 1
                h2_order = (1, 0) if last_pair else (0, 1)
                for h2 in h2_order:
                    h = 2 * pair + h2
                    rec = small.tile([HD + 1, N], F32, tag="rec", name="rec")
                    oraw = small.tile([HD, N], F32, tag="oraw", name="oraw")
                    rb = small.tile([HD, N], F32, tag="rb", name="rb")
                    for ih in range(2):
                        sl = slice(ih * 512, (ih + 1) * 512)
                        po = psO_pool.tile([HD + 1, 512], F32, tag="psO", name="po")
                        for jt in range(JT):
                            nc.tensor.matmul(
                                po,
                                lhsT=VT[jt][:, h, :],
                                rhs=p16_slice(P16, jt, h2, ih),
                                start=(jt == 0),
                                stop=(jt == JT - 1),
                            )
                        # softmax denominator sits in row HD of po
                        nc.vector.reciprocal(rec[HD : HD + 1, sl], po[HD : HD + 1, :])
                        # copy the raw head output out of PSUM immediately
                        # (frees the psO slot); on the last pair ScalarE is
                        # done with exps, so use it and keep DVE off the
                        # critical chain
                        if last_pair:
                            nc.scalar.copy(oraw[:, sl], po[0:HD, :])
                        else:
                            nc.vector.tensor_copy(oraw[:, sl], po[0:HD, :])
                        # per-half DRAM bounce broadcasts 1/colsum across
                        # partitions (SBUF APs reject 0 partition stride)
                        dmae = nc.scalar if last_pair else nc.sync
                        dmae.dma_start(
                            out=rdram[h : h + 1, sl], in_=rec[HD : HD + 1, sl]
                        )
                        rsrc = rdram[h : h + 1, sl]
                        nc.sync.dma_start(
                            out=rb[:, sl],
                            in_=bass.AP(
                                tensor=rsrc.tensor,
                                offset=rsrc.offset,
                                ap=[[0, HD]] + list(rsrc.ap[1:]),
                            ),
                        )
                    osc = None
                    if h2 != 0:
                        osc = small.tile([HD, N], BF16, tag="osc", name="osc")
                    for ih in range(2):
                        sl = slice(ih * 512, (ih + 1) * 512)
                        if h2 == 0:
                            nc.vector.tensor_tensor(
                                O16[pair][0:HD, sl], oraw[:, sl], rb[:, sl], AOP.mult
                            )
                        else:
                            nc.vector.tensor_tensor(
                                osc[:, sl], oraw[:, sl], rb[:, sl], AOP.mult
                            )
                            (nc.scalar if last_pair else nc.sync).dma_start(
                                out=O16[pair][HD:P, sl], in_=osc[:, sl]
                            )

            with tc.tile_pool(name="pp", bufs=4, space="PSUM") as pp:
                # ---------- input loads ----------
                x16r = x16d.rearrange("(t p) n -> t p n", p=P)
                xs16 = []
                for t in range(CT):
                    xt = persist.tile([P, N], BF16, tag=f"x16_{t}", name=f"x16_{t}")
                    nc.sync.dma_start(out=xt, in_=x16r[t])
                    xs16.append(xt)

                # interleave wq/wk tiles so K0's accumulation matmuls can
                # trickle-start alongside Q0's instead of waiting for the
                # whole of wq to finish on the same queue
                wqr = wqd.rearrange("(t p) o -> t p o", p=P)
                wkr = wkd.rearrange("(t p) o -> t p o", p=P)
                wqs, wks = [], []
                for t in range(CT):
                    wqt_ = persist.tile([P, DIM], BF16, tag=f"wq_{t}", name=f"wq_{t}")
                    nc.scalar.dma_start(out=wqt_, in_=wqr[t])
                    wqs.append(wqt_)
                    wkt_ = persist.tile([P, DIM], BF16, tag=f"wk_{t}", name=f"wk_{t}")
                    nc.scalar.dma_start(out=wkt_, in_=wkr[t])
                    wks.append(wkt_)
                bq_sb = load_b(bqd, "bq")
                bk_sb = load_b(bkd, "bk")

                # trigger the ~2.7us exp table load on ScalarE right after
                # its weight-DMA issues, so the first real exp doesn't pay it
                warm = small.tile([1, 8], F32, tag="warm", name="warm")
                nc.vector.memset(warm, 0.0)
                nc.scalar.activation(warm, warm, EXP)

                # ------ Q, K projections: [CT][128, N] bf16, [c, n] layout
                def project_one(ws, b_sb, name, ot):
                    qt = persist.tile(
                        [P, N], BF16, tag=f"{name}_{ot}", name=f"{name}_{ot}"
                    )
                    for nh in range(2):
                        ps = pp.tile(
                            [P, 512], F32, tag="pp", name=f"pp_{name}{ot}{nh}"
                        )
                        for ct in range(CT):
                            nc.tensor.matmul(
                                ps,
                                lhsT=ws[ct][:, ot * P : (ot + 1) * P],
                                rhs=xs16[ct][:, nh * 512 : (nh + 1) * 512],
                                start=(ct == 0),
                                stop=(ct == CT - 1),
                            )
                        nc.vector.tensor_scalar_add(
                            qt[:, nh * 512 : (nh + 1) * 512],
                            ps,
                            b_sb[:, ot : ot + 1],
                        )
                    return qt

                Q, K = [], []
                Q.append(project_one(wqs, bq_sb, "q", 0))
                K.append(project_one(wks, bk_sb, "k", 0))

                # pair 0's S^T + exp right away: gets ScalarE going while
                # the remaining projections stream on the PE
                P16_0 = s_phase(0)
                Q.append(project_one(wqs, bq_sb, "q", 1))
                K.append(project_one(wks, bk_sb, "k", 1))
                P16_1 = s_phase(1)

                # ------ V^T projection: VT[jt] = [128, NH, HD+1] bf16
                wvs = load_w(wvd, "wv")
                bvB = persist.tile([P, DIM], F32, tag="bvB", name="bvB")
                nc.gpsimd.dma_start(
                    out=bvB,
                    in_=bass.AP(
                        tensor=bvd[:].tensor, offset=0, ap=[[0, P], [1, DIM]]
                    ),
                )
                VT = []
                for jt in range(JT):
                    vt = persist.tile(
                        [P, NH, HD + 1], BF16, tag=f"vt_{jt}", name=f"vt_{jt}"
                    )
                    ps = pp.tile([P, 512], F32, tag="pp", name=f"pp_v{jt}")
                    for ct in range(CT):
                        nc.tensor.matmul(
                            ps,
                            lhsT=xs16[ct][:, jt * P : (jt + 1) * P],
                            rhs=wvs[ct],
                            start=(ct == 0),
                            stop=(ct == CT - 1),
                        )
                    nc.vector.tensor_tensor(
                        vt[:, :, 0:HD],
                        ps.rearrange("p (h d) -> p h d", h=NH),
                        bvB.rearrange("p (h d) -> p h d", h=NH),
                        AOP.add,
                    )
                    nc.vector.memset(vt[:, :, HD : HD + 1], 1.0)
                    VT.append(vt)

                for ot in range(2, CT):
                    Q.append(project_one(wqs, bq_sb, "q", ot))
                    K.append(project_one(wks, bk_sb, "k", ot))

            # ---------- attention (heads 2p / 2p+1 live on partitions
            # 0-63 / 64-127 of Q/K c-tile p); the AV-accumulator pool
            # reuses banks the projection pool just released
            O16 = [
                persist.tile([P, N], BF16, tag=f"o16_{t}", name=f"o16_{t}")
                for t in range(CT)
            ]
            rdram = dram.tile([NH, N], F32, tag="rdram", name="rdram")
            with tc.tile_pool(name="psO", bufs=4, space="PSUM") as psO_pool:
                av_phase(0, P16_0, psO_pool, O16, rdram)
                P16_2 = s_phase(2)
                av_phase(1, P16_1, psO_pool, O16, rdram)
                P16_3 = s_phase(3)
                av_phase(2, P16_2, psO_pool, O16, rdram)
                av_phase(3, P16_3, psO_pool, O16, rdram)

                # loads for the output projection (low priority; the DMA
                # queues have slack mid-kernel)
                wos = load_w(wod, "wo")
                bo_sb = load_b(bod, "bo")
                x32r = x32d.rearrange("(t p) n -> t p n", p=P)
                xs32 = []
                for t in range(CT):
                    xt32 = persist.tile(
                        [P, N], F32, tag=f"x32_{t}", name=f"x32_{t}"
                    )
                    nc.sync.dma_start(out=xt32, in_=x32r[t])
                    nc.vector.tensor_scalar_add(xt32, xt32, bo_sb[:, t : t + 1])
                    xs32.append(xt32)

            # ---------- output projection + residual. ot0/ot1 psum tiles
            # come from the psS pool (slots drained by pair-3 exps);
            # ot2/ot3 from a pool reusing the psO banks (drained by the
            # early PSUM copies) — all 24 ct0-2 matmuls can therefore run
            # while the last head's epilogue is still in flight.
            with tc.tile_pool(name="po3", bufs=2, space="PSUM") as po3:
                outr = outd.rearrange("(t p) n -> t p n", p=P)

                def op_pre(ot, pool=None):
                    # ct 0..2 accumulation: issuable while the last head
                    # pair (feeding O16[3]) is still in its epilogue
                    if pool is None:
                        ps = psS_pool.tile([P, N], F32, tag="psS", name=f"ps_o{ot}")
                    else:
                        ps = pool.tile([P, N], F32, tag="op34", name=f"ps_o{ot}")
                    for nh in range(2):
                        for ct in range(CT - 1):
                            nc.tensor.matmul(
                                ps[:, nh * 512 : (nh + 1) * 512],
                                lhsT=wos[ct][:, ot * P : (ot + 1) * P],
                                rhs=O16[ct][:, nh * 512 : (nh + 1) * 512],
                                start=(ct == 0),
                                stop=(ct == CT - 2),
                            )
                    return ps

                def op_post(ot, ps):
                    # ct 3 continues the accumulation in a second group,
                    # then bias+residual and writeback
                    for nh in range(2):
                        nc.tensor.matmul(
                            ps[:, nh * 512 : (nh + 1) * 512],
                            lhsT=wos[CT - 1][:, ot * P : (ot + 1) * P],
                            rhs=O16[CT - 1][:, nh * 512 : (nh + 1) * 512],
                            start=False,
                            stop=True,
                            skip_group_check=True,
                        )
                    for nh in range(2):
                        ob = otile.tile([P, 512], F32, tag="ob", name="ob")
                        nc.vector.tensor_tensor(
                            ob,
                            ps[:, nh * 512 : (nh + 1) * 512],
                            xs32[ot][:, nh * 512 : (nh + 1) * 512],
                            AOP.add,
                        )
                        nc.sync.dma_start(
                            out=outr[ot][:, nh * 512 : (nh + 1) * 512], in_=ob
                        )

                ps0 = op_pre(0)
                ps1 = op_pre(1)
                ps2 = op_pre(2, po3)
                ps3 = op_pre(3, po3)
                op_post(0, ps0)
                op_post(1, ps1)
                op_post(2, ps2)
                op_post(3, ps3)
    return nc


_BF = ml_dtypes.bfloat16


def _prep_maps(x, Wq, bq, Wk, bk, Wv, bv, Wo, bo):
    # plain numpy up front: inputs may arrive as jax device arrays and
    # transforming those would trigger on-device jax execution
    x, Wq, bq, Wk, bk, Wv, bv, Wo, bo = (
        np.asarray(a) for a in (x, Wq, bq, Wk, bk, Wv, bv, Wo, bo)
    )
    B, C, H, W = x.shape
    xf = np.ascontiguousarray(x.reshape(B, C, H * W)).astype(np.float32)
    shared = {
        "wqt": np.ascontiguousarray(Wq.T).astype(_BF),
        "wkt": np.ascontiguousarray(Wk.T).astype(_BF),
        "wvt": np.ascontiguousarray(Wv.T).astype(_BF),
        "wot": np.ascontiguousarray(Wo.T).astype(_BF),
        "bq": np.asarray(bq, np.float32),
        "bk": np.asarray(bk, np.float32),
        "bv": np.asarray(bv, np.float32),
        "bo": np.asarray(bo, np.float32),
    }
    in_maps = []
    for b in range(B):
        m = dict(shared)
        m["x32"] = xf[b]
        m["x16"] = xf[b].astype(_BF)
        in_maps.append(m)
    return in_maps


def kernel(x, Wq, bq, Wk, bk, Wv, bv, Wo, bo, _trace=False):
    from concourse.bass_utils import run_bass_kernel_spmd

    x = np.asarray(x)
    B, C, H, W = x.shape
    in_maps = _prep_maps(x, Wq, bq, Wk, bk, Wv, bv, Wo, bo)
    nc = build_nc()
    res = run_bass_kernel_spmd(nc, in_maps, core_ids=list(range(B)), trace=_trace)
    out = np.stack([res.results[b]["out"] for b in range(B)])
    out = out.reshape(B, C, H, W).astype(np.float32)
    if _trace:
        kernel.last_results = res
    return out



# revision 25
# speedup vs baseline: 1.2048x; 1.2048x over previous
"""Multi-head attention (dense_transformer) Trainium2 Bass kernel.

Problem: x[8, 512, 32, 32]; per-batch 1x1-conv QKV projections, 8-head
attention over N=H*W=1024 positions (head_dim 64), output projection,
residual. Sharding: data-parallel over batch B=8 across the 8 cores -
one batch element per core, no collectives.

v2: fp8e4 + DoubleRow matmuls everywhere (2x128 contraction per
instruction at 0.5 PE cycles/row -> 3x less PE time than bf16), softmax
exp split across the Activation engine (native Exp) and GPSIMD
(tensor_tensor pow with base e^(0.125/256), so the logit scale rides in
the base), and all bias work folded away:
  - bk dropped exactly (softmax is invariant to per-query logit shifts),
  - bq folded into the Q PSUM->SBUF cast (per-partition tensor_scalar),
  - bv folded via attention(V + bv) = attention(V) + bv into the
    residual: x32 = x + bo + Wo @ bv is prepared on the host,
  - weights are host-scaled by 16 to sit in fp8e4's sweet spot; the
    VT ones column (2^-6) and the final output scale 2^-14 undo it.

Per-core dataflow (all matmul operands fp8e4, fp32 PSUM accumulate):
  - Q8/K8 in DoubleRow layout [128, 2, N]: partition 32q+r, group g of
    head-group tile t holds channel o = 64*(4t + r//32) + 32g + (r%32)
    (host permutes Wq/Wk columns so the projection writes this layout
    directly); the same layout serves the S^T matmul with the d=64
    contraction as [32 partitions x 2 groups] at PE tile base 32q.
  - S^T per (head, jt): one DR matmul per 512 columns; exp (Act) or
    pow (Pool) -> P8 [128, jt, N] fp8.
  - AV: VT [128, jt, h, 64+ones] fp8; 4 DR matmuls accumulate [65, 512];
    row 64 = 2^-6 * colsum(P). Denominators bounce through DRAM per
    head ([2,512] -> [128,8] reciprocal -> broadcast [64,512]), then one
    tensor_tensor mult normalizes straight out of PSUM into O16 fp8.
  - Output projection: DR over O16 [128, 4, N]; epilogue is a single
    scalar_tensor_tensor (psum * 2^-14) + x32 per [128,512] tile.
"""

import sys

if "/opt/trn_rl_repo" not in sys.path:
    sys.path.insert(0, "/opt/trn_rl_repo")

import numpy as np
import ml_dtypes

import concourse.bass as bass
import concourse.mybir as mybir
from concourse.tile import TileContext

DIM = 512
NH = 8
HD = 64
N = 1024
P = 128
CT = DIM // P  # 4 c-tiles of 128 channels
JT = N // P    # 8 j-tiles of 128 positions
F32 = mybir.dt.float32
BF16 = mybir.dt.bfloat16
FP8 = mybir.dt.float8e4
AOP = mybir.AluOpType
EXP = mybir.ActivationFunctionType.Exp
DR = mybir.MatmulPerfMode.DoubleRow

WS = 16.0                      # host weight scale (fp8 range)
SEXP = 0.125 / (WS * WS)       # exp scale: 1/sqrt(64) / (16*16)
ONES_VAL = 2.0 ** -6           # denominator lhsT -> O16 = 1024*attn(V)
OUT_SCALE = 2.0 ** -14         # undo 16(Wo) * 1024(O16) / 16(V)... = 2^14

# Schraudolph exp on DVE: trunc(S*SCHRA_A + SCHRA_B) as int8 IS
# exp(S*SEXP) in fp8e4m3 bits (GPSIMD can't read PSUM; DVE has no exp -
# but one tensor_scalar mult+add into an int8 bitcast view is enough).
# B tuned for min mean |rel err| (2.6%) under truncation semantics.
SCHRA_A = float(8 * np.log2(np.e) * SEXP)
SCHRA_B = 55.58  # HW rounds the f32->int8 convert; CoreSim truncates

# exp engine pattern: 3 Act : 1 DVE-Schraudolph (DVE carries the rest of
# the elementwise work; Act only does exps)
ACT_PAT = (True, True, True, False)


class FixedTileContext(TileContext):
    """Works around a walrus/bass snapshot mismatch: this walrus build
    accepts only one sync-wait command per instruction, but Tile's wait
    assigner happily attaches several. After scheduling, excess waits on
    any instruction are peeled off onto same-engine NOPs inserted right
    before it (same blocking semantics: the engine executes in order)."""

    MAX_WAITS = 1
    MAX_WAITS_DATA = 1
    _wsplit_ctr = 0

    def _split_sync_waits(self):
        seq_only = mybir.SEQUENCER_ONLY_OPCODES
        for fn in self.nc.m.functions:
            for blk in fn.blocks:
                insts = list(blk.instructions)
                out = []
                for inst in insts:
                    si = inst.sync_info
                    limit = (
                        self.MAX_WAITS
                        if inst.opcode in seq_only
                        else self.MAX_WAITS_DATA
                    )
                    if si is not None and len(si.on_wait) > limit:
                        waits = list(si.on_wait)
                        movers = waits[:-limit]
                        keep = waits[-limit:]
                        del si.on_wait[:]
                        for w in keep:
                            si.on_wait.append(w)
                        for w in movers:
                            FixedTileContext._wsplit_ctr += 1
                            nop = mybir.InstNoOp(
                                name=f"wsplit-{FixedTileContext._wsplit_ctr}",
                                ins=[],
                                outs=[],
                            )
                            nop.engine = inst.engine
                            nop.sync_info = mybir.SyncInfo(on_wait=[w], on_update=[])
                            out.append(nop)
                    out.append(inst)
                if len(out) != len(insts):
                    del blk.instructions[:]
                    for i in out:
                        blk.add_instruction(i)

    split_on_exit = True

    def __exit__(self, *exc):
        ret = super().__exit__(*exc)
        if exc[0] is None and self.split_on_exit:
            self._split_sync_waits()
        return ret


def build_nc(split_waits=True):
    nc = bass.Bass()

    x8d = nc.dram_tensor("x8", [P, CT, N], FP8, kind="ExternalInput")
    x32d = nc.dram_tensor("x32", [DIM, N], F32, kind="ExternalInput")
    wqd = nc.dram_tensor("wq", [P, CT, DIM], FP8, kind="ExternalInput")
    wkd = nc.dram_tensor("wk", [P, CT, DIM], FP8, kind="ExternalInput")
    wvd = nc.dram_tensor("wv", [P, CT, DIM], FP8, kind="ExternalInput")
    wod = nc.dram_tensor("wo", [P, CT, DIM], FP8, kind="ExternalInput")
    bqd = nc.dram_tensor("bq", [P, CT], F32, kind="ExternalInput")
    outd = nc.dram_tensor("out", [DIM, N], F32, kind="ExternalOutput")

    FixedTileContext.split_on_exit = split_waits
    with FixedTileContext(nc) as tc:
        with (
            tc.tile_pool(name="persist", bufs=1) as persist,
            tc.tile_pool(name="p8pool", bufs=3) as p8pool,
            tc.tile_pool(name="small", bufs=4) as small,
            tc.tile_pool(name="rbpool", bufs=3) as rbpool,
            tc.tile_pool(name="otile", bufs=4) as otile,
            tc.tile_pool(name="dram", bufs=1, space="DRAM") as dram,
            tc.tile_pool(name="psS", bufs=2, space="PSUM") as psS_pool,
        ):
            # ---------- persistent SBUF tensors ----------
            x8_sb = persist.tile([P, CT, N], FP8, tag="x8", name="x8")
            wq_sb = persist.tile([P, CT, DIM], FP8, tag="wq", name="wq")
            wk_sb = persist.tile([P, CT, DIM], FP8, tag="wk", name="wk")
            wv_sb = persist.tile([P, CT, DIM], FP8, tag="wv", name="wv")
            wo_sb = persist.tile([P, CT, DIM], FP8, tag="wo", name="wo")
            bq_sb = persist.tile([P, CT], F32, tag="bq", name="bq")
            Q8 = [
                persist.tile([P, 2, N], FP8, tag=f"q8_{t}", name=f"q8_{t}")
                for t in range(2)
            ]
            K8 = [
                persist.tile([P, 2, N], FP8, tag=f"k8_{t}", name=f"k8_{t}")
                for t in range(2)
            ]
            VT = persist.tile([P, JT, NH, HD], FP8, tag="vt", name="vt")
            ones64 = persist.tile([P, 2, HD], FP8, tag="ones64", name="ones64")
            O16 = persist.tile([P, CT, N], FP8, tag="o16", name="o16")
            xs32 = [
                persist.tile([P, N], F32, tag=f"x32_{t}", name=f"x32_{t}")
                for t in range(CT)
            ]

            # ---------- input loads ----------
            # sync queue: x8 (needed first)
            nc.sync.dma_start(out=x8_sb, in_=x8d[:])
            # scalar queue (idle until first exp): Q/K weights + bias
            nc.scalar.dma_start(out=wq_sb, in_=wqd[:])
            nc.scalar.dma_start(out=wk_sb, in_=wkd[:])
            nc.scalar.dma_start(out=bq_sb, in_=bqd[:])
            # gpsimd queue (cheap issue): the rest
            nc.gpsimd.dma_start(out=wv_sb, in_=wvd[:])
            nc.gpsimd.dma_start(out=wo_sb, in_=wod[:])
            x32r = x32d.rearrange("(t p) n -> t p n", p=P)
            for t in range(CT):
                nc.gpsimd.dma_start(out=xs32[t], in_=x32r[t])

            # warm the exp table on Act; fill the pow base tile on Pool
            warm = small.tile([1, 8], F32, tag="warm", name="warm")
            nc.vector.memset(warm, 0.0)
            nc.scalar.activation(warm, warm, EXP)
            # denominator matmul lhsT (value 2^-6, exact in fp8): broadcasts
            # the P colsum across 64 output partitions
            nc.vector.memset(ones64, ONES_VAL)

            # ---------- exp unit emission (Act / Pool split) ----------
            exp_ctr = [0]

            def exp_unit(ps, p8t, jt):
                u = exp_ctr[0]
                exp_ctr[0] += 1
                if ACT_PAT[u % len(ACT_PAT)]:
                    nc.scalar.activation(p8t[:, jt, :], ps, EXP, scale=SEXP)
                else:
                    nc.vector.tensor_scalar(
                        p8t[:, jt, :].bitcast(mybir.dt.int8),
                        ps,
                        SCHRA_A,
                        SCHRA_B,
                        AOP.mult,
                        AOP.add,
                    )

            def s_head(h, p8t):
                """S^T + exp for head h -> P8 tile [P, JT, N]."""
                t, q = divmod(h, 4)
                b0 = 32 * q
                for jt in range(JT):
                    ps = psS_pool.tile([P, N], F32, tag="psS", name="psS")
                    for ih in range(2):
                        nc.tensor.matmul(
                            ps[:, ih * 512 : (ih + 1) * 512],
                            lhsT=K8[t][b0 : b0 + 32, :, jt * P : (jt + 1) * P],
                            rhs=Q8[t][b0 : b0 + 32, :, ih * 512 : (ih + 1) * 512],
                            start=True,
                            stop=True,
                            perf_mode=DR,
                            tile_position=(b0, 0),
                        )
                    exp_unit(ps, p8t, jt)

            def av_head(h, p8t, psO_pool):
                """AV + denominators + normalize for head h.

                DoubleRow matmuls may only target PSUM partition base 0, so
                each head gets its own [64, N] AV tile plus a [64, N]
                denominator tile where the ones64 matmul replicates the P
                colsum across all 64 output partitions (same free-size
                cost). reciprocal then goes PSUM->SBUF already in the shape
                the normalize mult wants - no DRAM bounce anywhere."""
                pr, hh = divmod(h, 2)
                po = psO_pool.tile([HD, N], F32, tag="psO", name="po")
                pd = psO_pool.tile([HD, N], F32, tag="psO", name="pd")
                for ih in range(2):
                    for jp in range(JT // 2):
                        nc.tensor.matmul(
                            po[:, ih * 512 : (ih + 1) * 512],
                            lhsT=VT[:, 2 * jp : 2 * jp + 2, h, :],
                            rhs=p8t[:, 2 * jp : 2 * jp + 2,
                                    ih * 512 : (ih + 1) * 512],
                            start=(jp == 0),
                            stop=(jp == JT // 2 - 1),
                            perf_mode=DR,
                            skip_group_check=True,
                        )
                    for jp in range(JT // 2):
                        nc.tensor.matmul(
                            pd[:, ih * 512 : (ih + 1) * 512],
                            lhsT=ones64,
                            rhs=p8t[:, 2 * jp : 2 * jp + 2,
                                    ih * 512 : (ih + 1) * 512],
                            start=(jp == 0),
                            stop=(jp == JT // 2 - 1),
                            perf_mode=DR,
                            skip_group_check=True,
                        )
                rb = rbpool.tile([HD, N], F32, tag="rb", name="rb")
                nc.vector.reciprocal(rb, pd)
                nc.vector.tensor_tensor(
                    O16[hh * HD : (hh + 1) * HD, pr, :], po, rb, AOP.mult
                )

            with tc.tile_pool(name="pp", bufs=2, space="PSUM") as pp:
                # ------ Q/K projections for head-group t: DR layout
                def project_qk(w_sb, dst, t, bias):
                    for g in range(2):
                        ps = pp.tile([P, N], F32, tag="pp", name="pp")
                        for nh in range(2):
                            for a in range(2):
                                nc.tensor.matmul(
                                    ps[:, nh * 512 : (nh + 1) * 512],
                                    lhsT=w_sb[
                                        :, 2 * a : 2 * a + 2,
                                        256 * t + 128 * g : 256 * t + 128 * g + 128,
                                    ],
                                    rhs=x8_sb[
                                        :, 2 * a : 2 * a + 2,
                                        nh * 512 : (nh + 1) * 512,
                                    ],
                                    start=(a == 0),
                                    stop=(a == 1),
                                    perf_mode=DR,
                                )
                        if bias is not None:
                            nc.vector.tensor_scalar_add(
                                dst[:, g, :],
                                ps,
                                bias[:, 2 * t + g : 2 * t + g + 1],
                            )
                        else:
                            nc.vector.tensor_copy(dst[:, g, :], ps)

                project_qk(wq_sb, Q8[0], 0, bq_sb)
                project_qk(wk_sb, K8[0], 0, None)

                # heads 0, 1 S+exp early: gets Act/Pool going while the
                # remaining projections stream on the PE
                P8 = {}
                P8[0] = p8pool.tile([P, JT, N], FP8, tag="p8", name="p8")
                s_head(0, P8[0])
                P8[1] = p8pool.tile([P, JT, N], FP8, tag="p8", name="p8")
                s_head(1, P8[1])

                project_qk(wq_sb, Q8[1], 1, bq_sb)
                project_qk(wk_sb, K8[1], 1, None)

                # ------ V projection -> VT [P, jt, h, d] (2 jt per psum buf)
                for jt in range(JT):
                    half = jt % 2
                    if half == 0:
                        psv = pp.tile([P, N], F32, tag="pp", name="ppv")
                    ps = psv[:, half * 512 : (half + 1) * 512]
                    for a in range(2):
                        nc.tensor.matmul(
                            ps,
                            lhsT=x8_sb[:, 2 * a : 2 * a + 2, jt * P : (jt + 1) * P],
                            rhs=wv_sb[:, 2 * a : 2 * a + 2, :],
                            start=(a == 0),
                            stop=(a == 1),
                            perf_mode=DR,
                        )
                    nc.vector.tensor_copy(
                        VT[:, jt, :, :],
                        ps.rearrange("p (h d) -> p h d", h=NH),
                    )

            # ---------- attention head pipeline ----------
            with tc.tile_pool(name="psO", bufs=2, space="PSUM") as psO_pool:
                def new_p8(h):
                    P8[h] = p8pool.tile([P, JT, N], FP8, tag="p8", name="p8")
                    s_head(h, P8[h])

                new_p8(2)
                av_head(0, P8[0], psO_pool)
                new_p8(3)
                av_head(1, P8[1], psO_pool)
                new_p8(4)
                av_head(2, P8[2], psO_pool)
                new_p8(5)
                av_head(3, P8[3], psO_pool)
                new_p8(6)
                av_head(4, P8[4], psO_pool)
                new_p8(7)
                av_head(5, P8[5], psO_pool)
                av_head(6, P8[6], psO_pool)
                av_head(7, P8[7], psO_pool)

            # ---------- output projection + residual ----------
            outr = outd.rearrange("(t p) n -> t p n", p=P)

            def out_proj(ot, ps):
                for nh in range(2):
                    for g in range(2):
                        nc.tensor.matmul(
                            ps[:, nh * 512 : (nh + 1) * 512],
                            lhsT=wo_sb[:, 2 * g : 2 * g + 2,
                                       ot * P : (ot + 1) * P],
                            rhs=O16[:, 2 * g : 2 * g + 2,
                                    nh * 512 : (nh + 1) * 512],
                            start=(g == 0),
                            stop=(g == 1),
                            perf_mode=DR,
                        )
                ob = otile.tile([P, N], F32, tag="ob", name="ob")
                nc.vector.scalar_tensor_tensor(
                    ob, ps, OUT_SCALE, xs32[ot], AOP.mult, AOP.add
                )
                nc.scalar.dma_start(out=outr[ot], in_=ob)

            with tc.tile_pool(name="po3", bufs=2, space="PSUM") as po3:
                for ot in range(2):
                    out_proj(ot, psS_pool.tile([P, N], F32, tag="psS",
                                               name=f"ps_o{ot}"))
                for ot in range(2, 4):
                    out_proj(ot, po3.tile([P, N], F32, tag="op34",
                                          name=f"ps_o{ot}"))
    return nc


_BF = ml_dtypes.bfloat16
_F8 = ml_dtypes.float8_e4m3


def _prep_maps(x, Wq, bq, Wk, bk, Wv, bv, Wo, bo):
    # plain numpy up front: inputs may arrive as jax device arrays and
    # transforming those would trigger on-device jax execution
    x, Wq, bq, Wk, bk, Wv, bv, Wo, bo = (
        np.asarray(a, np.float32) for a in (x, Wq, bq, Wk, bk, Wv, bv, Wo, bo)
    )
    B, C, H, W = x.shape
    xf = np.ascontiguousarray(x.reshape(B, C, H * W))
    rconst = bo + Wo @ bv  # residual constant: bo + Wo @ bv

    r_ = np.arange(P)
    cols = np.concatenate(
        [64 * (4 * t + r_ // 32) + 32 * g + (r_ % 32)
         for t in (0, 1) for g in (0, 1)]
    )

    def prep_qk(Wm):
        A = (WS * Wm)[cols, :].T  # [c, colpos]
        return np.ascontiguousarray(
            A.reshape(CT, P, DIM).transpose(1, 0, 2)
        ).astype(_F8)

    def prep_nat(Wm):
        A = (WS * Wm).T  # [c, o]
        return np.ascontiguousarray(
            A.reshape(CT, P, DIM).transpose(1, 0, 2)
        ).astype(_F8)

    shared = {
        "wq": prep_qk(Wq),
        "wk": prep_qk(Wk),
        "wv": prep_nat(Wv),
        "wo": prep_nat(Wo),
        "bq": np.ascontiguousarray(
            (WS * bq)[cols].reshape(CT, P).T
        ).astype(np.float32),
    }
    in_maps = []
    for b in range(B):
        m = dict(shared)
        m["x8"] = np.ascontiguousarray(
            xf[b].reshape(CT, P, N).transpose(1, 0, 2)
        ).astype(_F8)
        m["x32"] = xf[b] + rconst[:, None]
        in_maps.append(m)
    return in_maps


def kernel(x, Wq, bq, Wk, bk, Wv, bv, Wo, bo, _trace=False):
    from concourse.bass_utils import run_bass_kernel_spmd

    x = np.asarray(x)
    B, C, H, W = x.shape
    in_maps = _prep_maps(x, Wq, bq, Wk, bk, Wv, bv, Wo, bo)
    nc = build_nc()
    res = run_bass_kernel_spmd(nc, in_maps, core_ids=list(range(B)), trace=_trace)
    out = np.stack([res.results[b]["out"] for b in range(B)])
    out = out.reshape(B, C, H, W).astype(np.float32)
    if _trace:
        kernel.last_results = res
    return out
